# revision 13
# baseline (speedup 1.0000x reference)
"""Causal cross-attention kernel for 8 trn2 NeuronCores.

Sharding: 4-way data-parallel over batch x 2-way tensor-parallel over heads
(8 heads per core).  Each core computes, for its (batch, head-group):
projections Q^T/K^T/V, causal attention in transposed-score layout
(S^T[k,q] so the PV matmul contracts keys on partitions; the softmax
denominator comes from a ones-column appended to V), and a full-width
partial output projection.  The host sums the two head-group partials
per batch (the tensor-parallel all-reduce).

Single fully-interleaved device program: projection chunks, attention
q-waves and output-projection token blocks are emitted in proportionally
interleaved order so the PE always has independent matmul work queued
while the ACT engine runs the exp stream.  Everything is bf16 into the
matmuls with fp32 PSUM accumulation; softmax reciprocal broadcast runs
on the otherwise-idle GpSimd engine.
"""

import sys

sys.path.insert(0, "/opt/trn_rl_repo")

import numpy as np
import ml_dtypes

import concourse.bass as bass
import concourse.tile as tile
from concourse import bacc, mybir
from concourse.bass import ts
from concourse.masks import make_upper_triangular

F32 = mybir.dt.float32
BF16 = mybir.dt.bfloat16
P = 128

# full-problem constants
B_FULL = 4
S_FULL = 2048
D_FULL = 1024
HG_FULL = 8  # heads per core (16 heads / 2-way TP)
N_CORES = 8
PE_NS = 0.4167  # warm PE ns per row (for interleave proportions only)


def build_bass(S=S_FULL, D=D_FULL, HG=HG_FULL):
    """One-core program; SPMD across 8 cores with different data."""
    GO = HG * 64  # 512: output-feature width of this core's head group
    ND = D // P  # 8 contraction blocks
    NM = GO // P  # 4 o-tiles
    NQT = S // 512  # 4 q-waves
    NTB = S // P  # 16 token blocks
    NCH = S // 512  # 4 projection column chunks

    nc = bacc.Bacc("TRN2", target_bir_lowering=False, debug=False)
    xqT = nc.dram_tensor("xqT", [D, S], BF16, kind="ExternalInput")
    xkvT = nc.dram_tensor("xkvT", [D, S], BF16, kind="ExternalInput")
    wqT = nc.dram_tensor("wqT", [D, GO], BF16, kind="ExternalInput")
    wkT = nc.dram_tensor("wkT", [D, GO], BF16, kind="ExternalInput")
    wvT = nc.dram_tensor("wvT", [D, GO], BF16, kind="ExternalInput")
    woT = nc.dram_tensor("woT", [GO, D], BF16, kind="ExternalInput")
    y = nc.dram_tensor("y", [S, D], F32, kind="ExternalOutput")

    Exp = mybir.ActivationFunctionType.Exp

    with tile.TileContext(nc) as tc:
        from contextlib import ExitStack

        with ExitStack() as ctx:
            ctx.enter_context(
                nc.allow_low_precision(reason="bf16 matmul inputs, fp32 accumulate")
            )
            # ---- pools ----
            pers = ctx.enter_context(tc.tile_pool(name="pers", bufs=1))
            consts = ctx.enter_context(tc.tile_pool(name="consts", bufs=1))
            apool = ctx.enter_context(tc.tile_pool(name="pexp", bufs=4))
            rpool = ctx.enter_context(tc.tile_pool(name="rec", bufs=2))
            bpool = ctx.enter_context(tc.tile_pool(name="bc", bufs=2))
            ypool = ctx.enter_context(tc.tile_pool(name="ysb", bufs=3))
            pj_pool = ctx.enter_context(tc.tile_pool(name="pj", bufs=2, space="PSUM"))
            spool = ctx.enter_context(tc.tile_pool(name="pss", bufs=2, space="PSUM"))
            opool = ctx.enter_context(tc.tile_pool(name="pso", bufs=1, space="PSUM"))

            # ---- persistent SBUF ----
            xq = [pers.tile([P, S], BF16, tag=f"xq{d}", name=f"xq{d}") for d in range(ND)]
            xkv = [pers.tile([P, S], BF16, tag=f"xkv{d}", name=f"xkv{d}") for d in range(ND)]
            wq = [pers.tile([P, GO], BF16, tag=f"wq{d}", name=f"wq{d}") for d in range(ND)]
            wk = [pers.tile([P, GO], BF16, tag=f"wk{d}", name=f"wk{d}") for d in range(ND)]
            wv = [pers.tile([P, GO], BF16, tag=f"wv{d}", name=f"wv{d}") for d in range(ND)]
            wo = [pers.tile([P, D], BF16, tag=f"wo{m}", name=f"wo{m}") for m in range(NM)]
            qT = [pers.tile([P, S], BF16, tag=f"qT{m}", name=f"qT{m}") for m in range(NM)]
            kT = [pers.tile([P, S], BF16, tag=f"kT{m}", name=f"kT{m}") for m in range(NM)]
            oT = [pers.tile([P, S], BF16, tag=f"oT{m}", name=f"oT{m}") for m in range(NM)]
            vaug = [pers.tile([P, HG * 65], BF16, tag=f"va{t}", name=f"va{t}") for t in range(NTB)]

            # ---- consts; dummy exp hoists the ACT table load off the path ----
            dums = consts.tile([1, 2], F32)
            nc.vector.memset(dums[:], 0.0)
            dumo = consts.tile([1, 2], BF16)
            nc.scalar.activation(dumo[:], dums[:], Exp, scale=1.0)
            tri_f = consts.tile([P, P], F32)  # tri[k,q] = 1 if q >= k else 0
            make_upper_triangular(nc, tri_f[:], val=1.0, diag=True)
            tri = consts.tile([P, P], BF16)
            nc.vector.tensor_copy(tri[:], tri_f[:])
            # warm the PE p-state before the first real matmuls (cost model
            # halves PE speed for the first ~3us of activity)
            wrm = consts.tile([P, P], BF16)
            nc.vector.memset(wrm[:], 0.0)
            for i in range(0):
                pw = pj_pool.tile([P, 512], F32, tag="pj")
                nc.tensor.matmul(pw[:, 0:P], wrm[:], wrm[:], start=True, stop=True)

            # ---- DMA issue: SP lane (HWDGE) for wq/xq/y, Pool lane (SWDGE)
            # for the rest; order = first-needed-first on each lane ----
            for d in range(ND):
                # interleaved so QU(0,*) can accumulate d-blocks in arrival order
                nc.sync.dma_start(wq[d][:], wqT[ts(d, P), :])
                nc.sync.dma_start(xq[d][:, ts(0, 512)], xqT[ts(d, P), ts(0, 512)])
            for t in range(4):
                nc.gpsimd.memset(vaug[t][:], 1.0)
            for d in range(ND):
                nc.gpsimd.dma_start(xkv[d][:, ts(0, 512)], xkvT[ts(d, P), ts(0, 512)])
            for d in range(ND):
                nc.gpsimd.dma_start(wk[d][:], wkT[ts(d, P), :])
            for d in range(ND):
                nc.gpsimd.dma_start(wv[d][:], wvT[ts(d, P), :])
            for t in range(4, NTB):
                nc.gpsimd.memset(vaug[t][:], 1.0)
            for c in range(1, NCH):
                for d in range(ND):
                    nc.sync.dma_start(xq[d][:, ts(c, 512)], xqT[ts(d, P), ts(c, 512)])
                for d in range(ND):
                    nc.gpsimd.dma_start(xkv[d][:, ts(c, 512)], xkvT[ts(d, P), ts(c, 512)])
            for m in range(NM):
                nc.gpsimd.dma_start(wo[m][:], woT[ts(m, P), :])

            # ---- emission streams ----
            def proj_units(c):
                """Project chunk c of tokens: Q then K o-tiles, then V t-blocks."""
                for w_t, dst in ((wq, qT), (wk, kT)):
                    for m in range(NM):
                        pj = pj_pool.tile([P, 512], F32, tag="pj")
                        for d in range(ND):
                            nc.tensor.matmul(
                                pj[:],
                                w_t[d][:, ts(m, P)],
                                (xq if dst is qT else xkv)[d][:, ts(c, 512)],
                                start=(d == 0),
                                stop=(d == ND - 1),
                            )
                        nc.vector.tensor_copy(dst[m][:, ts(c, 512)], pj[:])
                        yield 8 * 512 * PE_NS
                for t2 in range(4):
                    tb = 4 * c + t2
                    pj = pj_pool.tile([P, 512], F32, tag="pj")
                    for d in range(ND):
                        nc.tensor.matmul(
                            pj[:],
                            xkv[d][:, ts(tb, P)],
                            wv[d][:],
                            start=(d == 0),
                            stop=(d == ND - 1),
                        )
                    nc.vector.tensor_copy(
                        vaug[tb][:].rearrange("p (h c) -> p h c", c=65)[:, :, 0:64],
                        pj[:].rearrange("p (h c) -> p h c", c=64),
                    )
                    yield 8 * 512 * PE_NS

            def attn_steps(qt):
                """Attention for q-wave qt, head pairs hp=0..3."""
                nkb = 4 * qt + 4
                for hp in range(NM):
                    psoA = opool.tile([65, 512], F32, tag="oA", name=f"oA{hp}_{qt}")
                    psoB = opool.tile([65, 512], F32, tag="oB", name=f"oB{hp}_{qt}")
                    for kb in range(nkb):
                        j = kb - 4 * qt
                        c0 = max(j, 0) * P
                        pss = spool.tile([P, 1024], F32, tag="pss", name=f"s{hp}_{qt}_{kb}")
                        for g, po in ((0, 0), (1, 64)):
                            nc.tensor.matmul(
                                pss[:, g * 512 + c0 : (g + 1) * 512],
                                kT[hp][po : po + 64, ts(kb, P)],
                                qT[hp][po : po + 64, qt * 512 + c0 : (qt + 1) * 512],
                                start=True,
                                stop=True,
                            )
                        pexp = apool.tile([P, 1024], BF16, tag="pexp", name=f"p{hp}_{qt}_{kb}")
                        nc.scalar.activation(
                            pexp[:].rearrange("p (g c) -> p g c", g=2)[:, :, c0:],
                            pss[:].rearrange("p (g c) -> p g c", g=2)[:, :, c0:],
                            Exp,
                            scale=0.125,
                        )
                        if j >= 0:
                            for g in (0, 1):
                                nc.vector.tensor_mul(
                                    pexp[:, g * 512 + c0 : g * 512 + c0 + P],
                                    pexp[:, g * 512 + c0 : g * 512 + c0 + P],
                                    tri[:],
                                )
                        for g, pso in ((0, psoA), (1, psoB)):
                            hh = 2 * hp + g
                            nc.tensor.matmul(
                                pso[:, c0:],
                                vaug[kb][:, hh * 65 : hh * 65 + 65],
                                pexp[:, g * 512 + c0 : (g + 1) * 512],
                                start=(kb == 0),
                                stop=(kb == nkb - 1),
                            )
                        yield 4 * (512 - c0) * PE_NS
                    # epilogue: 1/denominator, broadcast on GpSimd, scale into oT
                    for g, pso in ((0, psoA), (1, psoB)):
                        hh = 2 * hp + g
                        rec = rpool.tile([1, 512], BF16, tag="rec", name=f"rec{hh}_{qt}")
                        nc.vector.reciprocal(rec[:], pso[64:65, :])
                        bc = bpool.tile([64, 512], BF16, tag="bc", name=f"bc{hh}_{qt}")
                        nc.gpsimd.partition_broadcast(bc[:], rec[:])
                        row = hh * 64
                        nc.vector.tensor_mul(
                            oT[row // P][row % P : row % P + 64, ts(qt, 512)],
                            pso[0:64, :],
                            bc[:],
                        )

            def p3_units(mts):
                """Output projection for token blocks mts (full D width, partial
                over this head group; host adds the pair)."""
                for mt in mts:
                    for nt in range(D // 512):
                        pj = pj_pool.tile([P, 512], F32, tag="pj")
                        for ob in range(NM):
                            nc.tensor.matmul(
                                pj[:],
                                oT[ob][:, ts(mt, P)],
                                wo[ob][:, ts(nt, 512)],
                                start=(ob == 0),
                                stop=(ob == NM - 1),
                            )
                        ysb = ypool.tile([P, 512], F32, tag="ysb")
                        nc.vector.tensor_copy(ysb[:], pj[:])
                        nc.sync.dma_start(y[ts(mt, P), ts(nt, 512)], ysb[:])
                        yield 4 * 512 * PE_NS

            def p3_units_tail(mts):
                """Final-wave output projection: evictions alternate DVE/ACT and
                y stores alternate SP/Pool lanes so the drain pipelines."""
                Copy = mybir.ActivationFunctionType.Copy
                for i, (mt, nt) in enumerate((m, n) for m in mts for n in range(D // 512)):
                    if i % 2:
                        pjt = spool.tile([P, 1024], F32, tag="pss", name=f"yp{mt}_{nt}")
                        pj = pjt[:, 0:512]
                    else:
                        pjt = pj_pool.tile([P, 512], F32, tag="pj", name=f"yq{mt}_{nt}")
                        pj = pjt[:]
                    for ob in range(NM):
                        nc.tensor.matmul(
                            pj,
                            oT[ob][:, ts(mt, P)],
                            wo[ob][:, ts(nt, 512)],
                            start=(ob == 0),
                            stop=(ob == NM - 1),
                        )
                    ysb = ypool.tile([P, 512], F32, tag="ysb")
                    if i % 2:
                        nc.scalar.activation(ysb[:], pj, Copy, scale=1.0)
                    else:
                        nc.vector.tensor_copy(ysb[:], pj)
                    eng = nc.sync if i % 2 == 0 else nc.gpsimd
                    eng.dma_start(y[ts(mt, P), ts(nt, 512)], ysb[:])
                    yield 4 * 512 * PE_NS

            def drive(*gens_with_totals):
                """Proportional-progress interleave of emission streams."""
                items = [[g, float(t), 0.0, True] for g, t in gens_with_totals]
                while True:
                    live = [it for it in items if it[3]]
                    if not live:
                        break
                    it = min(live, key=lambda x: x[2] / x[1])
                    try:
                        it[2] += next(it[0])
                    except StopIteration:
                        it[3] = False

            PROJ_NS = 12 * 8 * 512 * PE_NS  # one chunk
            ATTN_NS = [
                sum(4 * (512 - max(kb - 4 * qt, 0) * P) * PE_NS for kb in range(4 * qt + 4)) * NM
                for qt in range(NQT)
            ]
            P3_NS = 2 * 4 * 512 * PE_NS  # one token block (both halves)

            drive((proj_units(0), PROJ_NS))
            drive((attn_steps(0), ATTN_NS[0]), (proj_units(1), PROJ_NS))
            drive(
                (attn_steps(1), ATTN_NS[1]),
                (proj_units(2), PROJ_NS),
                (p3_units(range(0, 4)), 4 * P3_NS),
            )
            drive((attn_steps(2), ATTN_NS[2]), (proj_units(3), PROJ_NS))
            # p3 total deflated so its units lag attention and fill the
            # final-epilogue latency window at the end of the wave
            drive((attn_steps(3), ATTN_NS[3]), (p3_units(range(4, 12)), 4 * P3_NS))
            drive((p3_units_tail(range(12, 16)), 4 * P3_NS))
    nc.finalize()
    return nc


_NC_CACHE = {}


def _get_nc():
    if "full" not in _NC_CACHE:
        _NC_CACHE["full"] = build_bass()
    return _NC_CACHE["full"]


def make_in_maps(query, key_value, Wq, Wk, Wv, Wo):
    bf16 = ml_dtypes.bfloat16
    query = np.asarray(query, dtype=np.float32)
    key_value = np.asarray(key_value, dtype=np.float32)
    Wq, Wk, Wv, Wo = (np.asarray(w, dtype=np.float32) for w in (Wq, Wk, Wv, Wo))
    GO = Wq.shape[0] // 2
    in_maps = []
    for c in range(N_CORES):
        b, g = c // 2, c % 2
        sl = slice(g * GO, (g + 1) * GO)
        in_maps.append(
            {
                "xqT": np.ascontiguousarray(query[b].T).astype(bf16),
                "xkvT": np.ascontiguousarray(key_value[b].T).astype(bf16),
                "wqT": np.ascontiguousarray(Wq[sl, :].T).astype(bf16),
                "wkT": np.ascontiguousarray(Wk[sl, :].T).astype(bf16),
                "wvT": np.ascontiguousarray(Wv[sl, :].T).astype(bf16),
                "woT": np.ascontiguousarray(Wo[:, sl].T).astype(bf16),
            }
        )
    return in_maps


def kernel(query, key_value, Wq, Wk, Wv, Wo):
    from concourse import bass_utils

    nc = _get_nc()
    in_maps = make_in_maps(query, key_value, Wq, Wk, Wv, Wo)
    res = bass_utils.run_bass_kernel_spmd(nc, in_maps, core_ids=list(range(N_CORES)))
    ys = [r["y"] for r in res.results]
    out = np.stack([ys[2 * b] + ys[2 * b + 1] for b in range(B_FULL)])
    return out.astype(np.float32)
